# revision 6
# baseline (speedup 1.0000x reference)
"""Trainium2 Bass kernel for nn_ContrastiveLoss (B=4, C=256, H=W=256).

Strategy
--------
The reference computes four families of per-position channel dot products
over columns of x viewed as [B, C, N] (N = H*W), then scalar reductions:

  fam1 (pos_sim): dot(x[:,:,pos[t]],  x[:,:,pos[t+P]])   t in [0,P)
  fam2 (neg_sim): dot(x[:,:,neg[t]],  x[:,:,neg[t+Ng]])  t in [0,Ng)
  fam3 (pn1):     dot(x[:,:,pos[t]],  x[:,:,neg[t]])     t in [0,M)
  fam4 (pn2):     dot(x[:,:,pos[t]],  x[:,:,neg[t]])     t in [M,2M)

Each column of x participates in at most two dot products, so the union of
the four pairings is a degree-<=2 graph = disjoint paths and even cycles.
The host walks those paths/cycles and emits columns in walk order; in the
permuted tensor xp every dot product is between ADJACENT columns.

Device pipeline (per core, S = E/8 edges, nb = S/512 blocks):
  - xp [B, C, S+1] fp16 streams into 8 resident SBUF tiles [128, S+1]
    (one per batch x channel-chunk), two DMAs each (split at the product
    slice boundary for finer pipelining).  Whole 16KB rows per descriptor
    keep the 16 DMA engines at peak (~26 B/ns each).
  - DVE computes shifted products p[i] = t[i]*t[i+1] in [128, 4096] slices
    at the 2x_1p rate (~2.3us each); two mid-stream slices go to GpSimd.
  - The tensor engine reduces products over C via a ones-staircase matmul:
    lhsT one-hot-column [128, nb] routes each 512-block's column-sum into
    PSUM row jj.  PSUM tiles are PER BATCH [nb, 512] (chunk 0/1
    accumulate), so each batch's d tensor completes as soon as its two
    tiles are processed and the scalar tail pipelines per batch.
  - Tail per batch (emitted one tile later to avoid head-of-line stalls on
    the DVE queue): ACT exp, then three fused scalar_tensor_tensor
    mask-multiply+accumulate ops produce (sum lin*d, sum m3*exp(d),
    sum m4*exp(d)) per PSUM row into rcat columns.
  - Final: ones-vector matmul column-sums rcat [nb, 12] -> [1, 12], copied
    and DMA'd out.  Host combines 8 cores x 4 batch groups.

exp() needs no max-subtraction: |d| < ~0.5 for this data regime, so
sum(exp(d)) is stable in fp32 (guarded by a finiteness assert on the host).
"""

import math
import sys

import numpy as np

if "/opt/trn_rl_repo" not in sys.path:  # harness runs from a fresh dir
    sys.path.insert(0, "/opt/trn_rl_repo")

B, C, N = 4, 256, 65536
N_CORES = 8
BLOCK = 512          # edges per PSUM block (= max fp32 matmul free dim)
CHUNKS = C // 128    # channel chunks of 128 partitions
UNIT_BLOCKS = 8      # 512-blocks per DVE product slice (4096 edges)


# ---------------------------------------------------------------- host prep

def _build_walk(y):
    """Column permutation + per-edge family labels (0 = junk/padding)."""
    y = np.asarray(y).reshape(-1)
    pos_idx = np.nonzero(y == 1)[0]
    neg_idx = np.nonzero(y == 0)[0]
    P = pos_idx.shape[0] // 2
    Ng = neg_idx.shape[0] // 2
    M = min(P, Ng)

    nP, nN = 2 * P, 2 * Ng
    V = nP + nN
    t_pos = np.arange(nP)
    t_neg = np.arange(nN)
    nbrA = np.empty(V, dtype=np.int64)
    nbrA[:nP] = np.where(t_pos < P, t_pos + P, t_pos - P)
    nbrA[nP:] = nP + np.where(t_neg < Ng, t_neg + Ng, t_neg - Ng)
    famA = np.empty(V, dtype=np.int8)
    famA[:nP] = 1
    famA[nP:] = 2
    nbrB = np.full(V, -1, dtype=np.int64)
    nbrB[:2 * M] = nP + t_pos[:2 * M]
    nbrB[nP:nP + 2 * M] = t_neg[:2 * M]
    famB = np.zeros(V, dtype=np.int8)
    famB[:M] = 3
    famB[M:2 * M] = 4
    famB[nP:nP + M] = 3
    famB[nP + M:nP + 2 * M] = 4

    visited = np.zeros(V, dtype=bool)
    order = np.empty(V + V // 4 + 16, dtype=np.int64)
    fams_l = np.empty(order.shape[0], dtype=np.int8)
    no = 0
    ne = 0

    def walk_from(v0, is_cycle):
        nonlocal no, ne
        if no > 0:
            fams_l[ne] = 0  # junk edge joining the previous component
            ne += 1
        v = v0
        use_A = True  # endpoints/cycle starts leave via their A edge
        order[no] = v
        no += 1
        visited[v] = True
        while True:
            if use_A:
                nxt, fam = nbrA[v], famA[v]
            else:
                nxt = nbrB[v]
                if nxt < 0:
                    return
                fam = famB[v]
            if visited[nxt]:
                if is_cycle and nxt == v0 and not use_A:
                    fams_l[ne] = fam
                    ne += 1
                    order[no] = v0  # close the cycle
                    no += 1
                return
            fams_l[ne] = fam
            ne += 1
            order[no] = nxt
            no += 1
            visited[nxt] = True
            v = nxt
            use_A = not use_A

    for v0 in np.nonzero(nbrB < 0)[0]:
        if not visited[v0]:
            walk_from(int(v0), is_cycle=False)
    for v0 in range(V):
        if not visited[v0]:
            walk_from(int(v0), is_cycle=True)

    n_real = int((fams_l[:ne] > 0).sum())
    assert n_real == P + Ng + 2 * M, (n_real, P + Ng + 2 * M)

    per = N_CORES * BLOCK
    E_pad = ((ne + per - 1) // per) * per
    V_pad = E_pad + 1
    fams = np.zeros(E_pad, dtype=np.int8)
    fams[:ne] = fams_l[:ne]
    vert = np.zeros(V_pad, dtype=np.int64)
    vert[:no] = order[:no]
    colmap = np.where(vert < nP, pos_idx[np.minimum(vert, nP - 1)],
                      neg_idx[np.maximum(vert - nP, 0)])
    return colmap, fams, P, Ng, M


def _units(nb):
    """Product slices: (col_start, n_cols) covering S in 4096-col chunks."""
    out = []
    for s in range(0, nb, UNIT_BLOCKS):
        w = min(UNIT_BLOCKS, nb - s) * BLOCK
        out.append((s * BLOCK, w))
    return out


# ------------------------------------------------------------- device program

def trace_program(nc, tc, ctx, S, nb, dt_in,
                  gp_units=((2, 1), (5, 1)), prod_bufs=4, tail_delay=1):
    """Emit the per-core program.

    DRAM (per core): xp [B, C, S+1] dt_in, msk [nb, 3*BLOCK] f32,
    out [1, 3*B] f32 = per-batch (sum lin*d, sum m3*e^d, sum m4*e^d).
    """
    import concourse.mybir as mybir

    f32 = mybir.dt.float32
    Alu = mybir.AluOpType
    Act = mybir.ActivationFunctionType
    R = nb
    units = _units(nb)

    xp = nc.dram_tensor("xp", [B, C, S + 1], dt_in, kind="ExternalInput").ap()
    msk = nc.dram_tensor("msk", [nb, 3 * BLOCK], f32, kind="ExternalInput").ap()
    out = nc.dram_tensor("out", [1, 3 * B], f32, kind="ExternalOutput").ap()

    const_pool = ctx.enter_context(tc.tile_pool(name="const", bufs=1))
    xp_pool = ctx.enter_context(tc.tile_pool(name="xp", bufs=1))
    prod_pool = ctx.enter_context(tc.tile_pool(name="prod", bufs=prod_bufs))
    stat_pool = ctx.enter_context(tc.tile_pool(name="stat", bufs=1))
    psum_pool = ctx.enter_context(tc.tile_pool(name="psum", bufs=1, space="PSUM"))

    # Staircase selector: zo[:, R-1] = 1 (all partitions), else 0.
    # lhsT = zo[:, R-1-r : 2R-1-r] puts the ones-column at local col r, so
    # the matmul lands the 128-partition column-sum in PSUM row r (other
    # rows accumulate exact zeros) -- matmul PSUM outputs must start at
    # partition 0, so rows can't be addressed via the output AP.
    zo = const_pool.tile([128, 2 * R - 1], dt_in)
    nc.vector.memset(zo[:], 0.0)
    nc.vector.memset(zo[:, R - 1:R], 1.0)
    ones_f32 = const_pool.tile([R, 1], f32)
    nc.vector.memset(ones_f32[:], 1.0)

    msk_t = const_pool.tile([R, 3 * BLOCK], f32)

    d_psums = [psum_pool.tile([R, BLOCK], f32, tag=f"d{b}", name=f"d_psum{b}")
               for b in range(B)]
    rcat = stat_pool.tile([R, 3 * B], f32)

    tiles = [(b, ch) for b in range(B) for ch in range(CHUNKS)]

    def tail(b):
        # keep the DVE free for products: ACT stages d into SBUF, GpSimd
        # applies the per-edge mask (multiplicative for the linear term,
        # additive -40 bias for the exp terms so junk edges vanish), and
        # ACT's free-axis accumulator does all three row reductions.
        d_sb = stat_pool.tile([R, BLOCK], f32, tag=f"d_sb{b % 2}")
        db3 = stat_pool.tile([R, BLOCK], f32, tag=f"db3_{b % 2}")
        db4 = stat_pool.tile([R, BLOCK], f32, tag=f"db4_{b % 2}")
        scrL = stat_pool.tile([R, BLOCK], f32, tag=f"scrL{b % 2}")
        junk = stat_pool.tile([R, BLOCK], f32, tag=f"junk{b % 2}")
        nc.scalar.copy(d_sb[:], d_psums[b][:])
        nc.gpsimd.tensor_mul(scrL[:], d_sb[:], msk_t[:, 0:BLOCK])
        nc.gpsimd.tensor_add(db3[:], d_sb[:], msk_t[:, BLOCK:2 * BLOCK])
        nc.gpsimd.tensor_add(db4[:], d_sb[:], msk_t[:, 2 * BLOCK:3 * BLOCK])
        nc.scalar.activation(junk[:], scrL[:], Act.Copy,
                             accum_out=rcat[:, 3 * b:3 * b + 1])
        nc.scalar.activation(junk[:], db3[:], Act.Exp,
                             accum_out=rcat[:, 3 * b + 1:3 * b + 2])
        nc.scalar.activation(junk[:], db4[:], Act.Exp,
                             accum_out=rcat[:, 3 * b + 2:3 * b + 3])

    pending = []  # batches whose tail is not yet emitted
    for i, (b, ch) in enumerate(tiles):
        t = xp_pool.tile([128, S + 1], dt_in, tag=f"xp{i}")
        for k, (cs, w) in enumerate(units):
            lo = cs + (1 if k > 0 else 0)
            nc.sync.dma_start(t[:, lo:cs + w + 1],
                              xp[b, 128 * ch:128 * (ch + 1), lo:cs + w + 1])
        if i == 0:
            nc.sync.dma_start(msk_t[:], msk)  # after first xp triggers
        for k, (cs, w) in enumerate(units):
            eng = nc.gpsimd if (i, k) in gp_units else nc.vector
            p = prod_pool.tile([128, w], dt_in)
            eng.tensor_mul(p[:], t[:, cs:cs + w], t[:, cs + 1:cs + w + 1])
            for jj in range(cs // BLOCK, (cs + w) // BLOCK):
                nc.tensor.matmul(
                    d_psums[b][:, :], zo[:, R - 1 - jj:2 * R - 1 - jj],
                    p[:, BLOCK * jj - cs:BLOCK * (jj + 1) - cs],
                    start=(ch == 0 and cs == 0 and jj == 0),
                    stop=(ch == CHUNKS - 1 and jj == nb - 1))
        if ch == CHUNKS - 1:
            pending.append(b)
        # emit tails tail_delay tiles after their stop matmul
        while pending and 2 * pending[0] + 1 + tail_delay <= i:
            tail(pending.pop(0))
    for b in pending:
        tail(b)

    f_psum = psum_pool.tile([1, 3 * B], f32, tag="final")
    nc.tensor.matmul(f_psum[:], ones_f32[:], rcat[:], start=True, stop=True)
    res = stat_pool.tile([1, 3 * B], f32)
    nc.scalar.copy(res[:], f_psum[:])
    nc.sync.dma_start(out, res[:])


_CACHE = {}


def _compiled(S, nb, dt_name, prog_opts=None):
    key = (S, nb, dt_name, str(sorted((prog_opts or {}).items(),
                                      key=lambda kv: kv[0])))
    if key in _CACHE:
        return _CACHE[key]
    from contextlib import ExitStack

    import concourse.bacc as bacc
    import concourse.mybir as mybir
    import concourse.tile as tile

    dt_in = getattr(mybir.dt, dt_name)
    nc = bacc.Bacc("TRN2", target_bir_lowering=False, debug=False,
                   num_devices=N_CORES)
    with tile.TileContext(nc) as tc:
        with ExitStack() as ctx:
            trace_program(nc, tc, ctx, S, nb, dt_in, **(prog_opts or {}))
    nc.compile()
    _CACHE[key] = nc
    return nc


# -------------------------------------------------------------------- kernel

def kernel(x, y, _dt_name="float16", _run_opts=None, _prog_opts=None):
    x = np.asarray(x)
    y = np.asarray(y)
    assert x.shape == (B, C, 256, 256) and y.shape == (N,)

    colmap, fams, P, Ng, M = _build_walk(y)
    E = fams.shape[0]
    S = E // N_CORES
    nb = S // BLOCK
    assert nb * BLOCK * N_CORES == E and nb <= 128

    np_dt = {"float32": np.float32, "float16": np.float16,
             "bfloat16": np.float16}[_dt_name]
    x3 = x.reshape(B, C, N)
    xp = np.ascontiguousarray(x3[:, :, colmap], dtype=np_dt)  # [B, C, E+1]

    # per-core masks in d-row layout: row jj covers edges i*S+jj*512+[0,512)
    fams_c = fams.reshape(N_CORES, nb, BLOCK)
    lin = np.where(fams_c == 1, -1.0 / (B * P),
                   np.where(fams_c == 2, -1.0 / (B * Ng), 0.0)).astype(np.float32)
    b3 = np.where(fams_c == 3, 0.0, -40.0).astype(np.float32)  # exp bias
    b4 = np.where(fams_c == 4, 0.0, -40.0).astype(np.float32)
    msk = np.concatenate([lin, b3, b4], axis=2)  # [N_CORES, nb, 3*BLOCK]

    in_maps = [
        {"xp": np.ascontiguousarray(xp[:, :, i * S:(i + 1) * S + 1]),
         "msk": msk[i]}
        for i in range(N_CORES)
    ]

    nc = _compiled(S, nb, _dt_name, _prog_opts)
    from concourse.bass_utils import run_bass_kernel_spmd

    res = run_bass_kernel_spmd(nc, in_maps, list(range(N_CORES)),
                               **(_run_opts or {}))
    partials = np.stack([r["out"][0] for r in res.results])  # [N_CORES, 3B]
    per = partials.sum(axis=0, dtype=np.float64).reshape(B, 3)
    s_lin, s3, s4 = per.sum(axis=0)

    n = float(B * M)
    loss = s_lin + math.log(s3) - math.log(n) + math.log(s4) - math.log(n)
    assert np.isfinite(loss)
    out = np.float32(loss)
    if _run_opts:
        return out, res
    return out


# revision 9
# speedup vs baseline: 1.2744x; 1.2744x over previous
"""Trainium2 Bass kernel for nn_ContrastiveLoss (B=4, C=256, H=W=256).

Strategy
--------
The reference computes four families of per-position channel dot products
over columns of x viewed as [B, C, N] (N = H*W), then scalar reductions:

  fam1 (pos_sim): dot(x[:,:,pos[t]],  x[:,:,pos[t+P]])   t in [0,P)
  fam2 (neg_sim): dot(x[:,:,neg[t]],  x[:,:,neg[t+Ng]])  t in [0,Ng)
  fam3 (pn1):     dot(x[:,:,pos[t]],  x[:,:,neg[t]])     t in [0,M)
  fam4 (pn2):     dot(x[:,:,pos[t]],  x[:,:,neg[t]])     t in [M,2M)

Each column of x participates in at most two dot products, so the union of
the four pairings is a degree-<=2 graph = disjoint paths and even cycles.
The host walks those paths/cycles and emits columns in walk order; in the
permuted tensor xp every dot product is between ADJACENT columns.

Device pipeline (per core, S = E/8 edges, nb = S/512 blocks):
  - xp [B, C, S+1] fp16 streams into 8 resident SBUF tiles [128, S+1]
    (one per batch x channel-chunk), two DMAs each (split at the product
    slice boundary for finer pipelining).  Whole 16KB rows per descriptor
    keep the 16 DMA engines at peak (~26 B/ns each).
  - DVE computes shifted products p[i] = t[i]*t[i+1] in [128, 4096] slices
    at the 2x_1p rate (~2.3us each); two mid-stream slices go to GpSimd.
  - The tensor engine reduces products over C via a ones-staircase matmul:
    lhsT one-hot-column [128, nb] routes each 512-block's column-sum into
    PSUM row jj.  PSUM tiles are PER BATCH [nb, 512] (chunk 0/1
    accumulate), so each batch's d tensor completes as soon as its two
    tiles are processed and the scalar tail pipelines per batch.
  - Tail per batch (emitted one tile later to avoid head-of-line stalls on
    the DVE queue): ACT exp, then three fused scalar_tensor_tensor
    mask-multiply+accumulate ops produce (sum lin*d, sum m3*exp(d),
    sum m4*exp(d)) per PSUM row into rcat columns.
  - Final: ones-vector matmul column-sums rcat [nb, 12] -> [1, 12], copied
    and DMA'd out.  Host combines 8 cores x 4 batch groups.

exp() needs no max-subtraction: |d| < ~0.5 for this data regime, so
sum(exp(d)) is stable in fp32 (guarded by a finiteness assert on the host).
"""

import math
import sys

import numpy as np

if "/opt/trn_rl_repo" not in sys.path:  # harness runs from a fresh dir
    sys.path.insert(0, "/opt/trn_rl_repo")

B, C, N = 4, 256, 65536
N_CORES = 8
BLOCK = 512          # edges per PSUM block (= max fp32 matmul free dim)
CHUNKS = C // 128    # channel chunks of 128 partitions
UNIT_BLOCKS = 8      # 512-blocks per DVE product slice (4096 edges)


# ---------------------------------------------------------------- host prep

def _build_walk(y):
    """Column permutation + per-edge family labels (0 = junk/padding)."""
    y = np.asarray(y).reshape(-1)
    pos_idx = np.nonzero(y == 1)[0]
    neg_idx = np.nonzero(y == 0)[0]
    P = pos_idx.shape[0] // 2
    Ng = neg_idx.shape[0] // 2
    M = min(P, Ng)

    nP, nN = 2 * P, 2 * Ng
    V = nP + nN
    t_pos = np.arange(nP)
    t_neg = np.arange(nN)
    nbrA = np.empty(V, dtype=np.int64)
    nbrA[:nP] = np.where(t_pos < P, t_pos + P, t_pos - P)
    nbrA[nP:] = nP + np.where(t_neg < Ng, t_neg + Ng, t_neg - Ng)
    famA = np.empty(V, dtype=np.int8)
    famA[:nP] = 1
    famA[nP:] = 2
    nbrB = np.full(V, -1, dtype=np.int64)
    nbrB[:2 * M] = nP + t_pos[:2 * M]
    nbrB[nP:nP + 2 * M] = t_neg[:2 * M]
    famB = np.zeros(V, dtype=np.int8)
    famB[:M] = 3
    famB[M:2 * M] = 4
    famB[nP:nP + M] = 3
    famB[nP + M:nP + 2 * M] = 4

    visited = np.zeros(V, dtype=bool)
    order = np.empty(V + V // 4 + 16, dtype=np.int64)
    fams_l = np.empty(order.shape[0], dtype=np.int8)
    no = 0
    ne = 0

    def walk_from(v0, is_cycle):
        nonlocal no, ne
        if no > 0:
            fams_l[ne] = 0  # junk edge joining the previous component
            ne += 1
        v = v0
        use_A = True  # endpoints/cycle starts leave via their A edge
        order[no] = v
        no += 1
        visited[v] = True
        while True:
            if use_A:
                nxt, fam = nbrA[v], famA[v]
            else:
                nxt = nbrB[v]
                if nxt < 0:
                    return
                fam = famB[v]
            if visited[nxt]:
                if is_cycle and nxt == v0 and not use_A:
                    fams_l[ne] = fam
                    ne += 1
                    order[no] = v0  # close the cycle
                    no += 1
                return
            fams_l[ne] = fam
            ne += 1
            order[no] = nxt
            no += 1
            visited[nxt] = True
            v = nxt
            use_A = not use_A

    for v0 in np.nonzero(nbrB < 0)[0]:
        if not visited[v0]:
            walk_from(int(v0), is_cycle=False)
    for v0 in range(V):
        if not visited[v0]:
            walk_from(int(v0), is_cycle=True)

    n_real = int((fams_l[:ne] > 0).sum())
    assert n_real == P + Ng + 2 * M, (n_real, P + Ng + 2 * M)

    per = N_CORES * BLOCK
    E_pad = ((ne + per - 1) // per) * per
    V_pad = E_pad + 1
    fams = np.zeros(E_pad, dtype=np.int8)
    fams[:ne] = fams_l[:ne]
    vert = np.zeros(V_pad, dtype=np.int64)
    vert[:no] = order[:no]
    colmap = np.where(vert < nP, pos_idx[np.minimum(vert, nP - 1)],
                      neg_idx[np.maximum(vert - nP, 0)])
    return colmap, fams, P, Ng, M


def _units(nb):
    """Product slices: (col_start, n_cols) covering S in 4096-col chunks."""
    out = []
    for s in range(0, nb, UNIT_BLOCKS):
        w = min(UNIT_BLOCKS, nb - s) * BLOCK
        out.append((s * BLOCK, w))
    return out


# ------------------------------------------------------------- device program

def trace_program(nc, tc, ctx, S, nb, dt_in,
                  gp_units=(), prod_bufs=4):
    """Emit the per-core program.

    DRAM (per core): xp [B, C, S+1] dt_in; mlin [R, BLOCK] f32 (linear
    weights); mbias [R, 2*BLOCK] dt_in (additive exp biases b3 | b4-b3);
    idn [R, R] dt_in (identity); out [1, 3] f32 = (sum lin*(d+b3),
    sum e^(d+b3), sum e^(d+b4)).
    """
    import concourse.mybir as mybir

    f32 = mybir.dt.float32
    Alu = mybir.AluOpType
    Act = mybir.ActivationFunctionType
    R = B * nb  # d row layout: r = nb*b + jj
    units = _units(nb)

    xp = nc.dram_tensor("xp", [B, C, S + 1], dt_in, kind="ExternalInput").ap()
    mlin = nc.dram_tensor("mlin", [R, BLOCK], f32, kind="ExternalInput").ap()
    mbias = nc.dram_tensor("mbias", [R, 2 * BLOCK], dt_in,
                           kind="ExternalInput").ap()
    idn = nc.dram_tensor("idn", [R, R], dt_in, kind="ExternalInput").ap()
    out = nc.dram_tensor("out", [1, 3], f32, kind="ExternalOutput").ap()

    const_pool = ctx.enter_context(tc.tile_pool(name="const", bufs=1))
    xp_pool = ctx.enter_context(tc.tile_pool(name="xp", bufs=1))
    prod_pool = ctx.enter_context(tc.tile_pool(name="prod", bufs=prod_bufs))
    stat_pool = ctx.enter_context(tc.tile_pool(name="stat", bufs=1))
    psum_pool = ctx.enter_context(tc.tile_pool(name="psum", bufs=1, space="PSUM"))

    # Staircase selector: zo[:, R-1] = 1 (all partitions), else 0.
    # lhsT = zo[:, R-1-r : 2R-1-r] puts the ones-column at local col r, so
    # the matmul lands the 128-partition column-sum in PSUM row r (other
    # rows accumulate exact zeros) -- matmul PSUM outputs must start at
    # partition 0, so rows can't be addressed via the output AP.
    zo = const_pool.tile([128, 2 * R - 1], dt_in)
    nc.vector.memset(zo[:], 0.0)
    nc.vector.memset(zo[:, R - 1:R], 1.0)
    ones_f32 = const_pool.tile([R, 1], f32)
    nc.vector.memset(ones_f32[:], 1.0)

    mlin_t = const_pool.tile([R, BLOCK], f32)
    mbias_t = const_pool.tile([R, 2 * BLOCK], dt_in)
    idn_t = const_pool.tile([R, R], dt_in)

    d_psum = psum_pool.tile([R, BLOCK], f32, tag="d")
    rcat = stat_pool.tile([R, 3], f32)

    tiles = [(b, ch) for b in range(B) for ch in range(CHUNKS)]
    n_mm = len(tiles) * nb
    i_mm = 0
    for i, (b, ch) in enumerate(tiles):
        t = xp_pool.tile([128, S + 1], dt_in, tag=f"xp{i}")
        for k, (cs, w) in enumerate(units):
            lo = cs + (1 if k > 0 else 0)
            nc.sync.dma_start(t[:, lo:cs + w + 1],
                              xp[b, 128 * ch:128 * (ch + 1), lo:cs + w + 1])
        if i == 0:  # small aux loads after the first xp triggers
            nc.sync.dma_start(mlin_t[:], mlin)
            nc.sync.dma_start(mbias_t[:], mbias)
            nc.sync.dma_start(idn_t[:], idn)
        for k, (cs, w) in enumerate(units):
            eng = nc.gpsimd if (i, k) in gp_units else nc.vector
            p = prod_pool.tile([128, w], dt_in)
            eng.tensor_mul(p[:], t[:, cs:cs + w], t[:, cs + 1:cs + w + 1])
            for jj in range(cs // BLOCK, (cs + w) // BLOCK):
                r = nb * b + jj
                nc.tensor.matmul(
                    d_psum[:, :], zo[:, R - 1 - r:2 * R - 1 - r],
                    p[:, BLOCK * jj - cs:BLOCK * (jj + 1) - cs],
                    start=(i_mm == 0), stop=False)
                i_mm += 1

    # tail: linear STT on the pure d first (the bias matmuls below mutate
    # d in PSUM), then d += b3 via identity-weighted matmul and
    # exp-accumulate on ACT; then d += (b4-b3) and exp-accumulate again.
    scr = stat_pool.tile([R, BLOCK], f32)
    junk = stat_pool.tile([R, BLOCK], f32)
    nc.vector.scalar_tensor_tensor(
        scr[:], d_psum[:], 1.0, mlin_t[:],
        Alu.mult, Alu.mult, accum_out=rcat[:, 0:1])
    nc.tensor.matmul(d_psum[:, :], idn_t[:], mbias_t[:, 0:BLOCK],
                     start=False, stop=False)
    nc.scalar.activation(junk[:], d_psum[:], Act.Exp,
                         accum_out=rcat[:, 1:2])
    nc.tensor.matmul(d_psum[:, :], idn_t[:], mbias_t[:, BLOCK:2 * BLOCK],
                     start=False, stop=True)
    nc.scalar.activation(junk[:], d_psum[:], Act.Exp,
                         accum_out=rcat[:, 2:3])

    f_psum = psum_pool.tile([1, 3], f32, tag="final")
    nc.tensor.matmul(f_psum[:], ones_f32[:], rcat[:], start=True, stop=True)
    res = stat_pool.tile([1, 3], f32)
    nc.scalar.copy(res[:], f_psum[:])
    nc.sync.dma_start(out, res[:])


_CACHE = {}


def _compiled(S, nb, dt_name, prog_opts=None):
    key = (S, nb, dt_name, str(sorted((prog_opts or {}).items(),
                                      key=lambda kv: kv[0])))
    if key in _CACHE:
        return _CACHE[key]
    from contextlib import ExitStack

    import concourse.bacc as bacc
    import concourse.mybir as mybir
    import concourse.tile as tile

    dt_in = getattr(mybir.dt, dt_name)
    nc = bacc.Bacc("TRN2", target_bir_lowering=False, debug=False,
                   num_devices=N_CORES)
    with tile.TileContext(nc) as tc:
        with ExitStack() as ctx:
            trace_program(nc, tc, ctx, S, nb, dt_in, **(prog_opts or {}))
    nc.compile()
    _CACHE[key] = nc
    return nc


# -------------------------------------------------------------------- kernel

def kernel(x, y, _dt_name="float16", _run_opts=None, _prog_opts=None):
    x = np.asarray(x)
    y = np.asarray(y)
    assert x.shape == (B, C, 256, 256) and y.shape == (N,)

    colmap, fams, P, Ng, M = _build_walk(y)
    E = fams.shape[0]
    S = E // N_CORES
    nb = S // BLOCK
    assert nb * BLOCK * N_CORES == E and nb <= 128

    np_dt = {"float32": np.float32, "float16": np.float16,
             "bfloat16": np.float16}[_dt_name]
    x3 = x.reshape(B, C, N)
    xp = np.ascontiguousarray(x3[:, :, colmap], dtype=np_dt)  # [B, C, E+1]

    # per-core masks in d-row layout: row nb*b+jj covers edges
    # i*S+jj*512+[0,512), identical across the B batch rows
    R = B * nb
    fams_c = fams.reshape(N_CORES, nb, BLOCK)
    lin = np.where(fams_c == 1, -1.0 / (B * P),
                   np.where(fams_c == 2, -1.0 / (B * Ng), 0.0)).astype(np.float32)
    b3 = np.where(fams_c == 3, 0.0, -40.0).astype(np_dt)  # additive exp bias
    b4 = np.where(fams_c == 4, 0.0, -40.0).astype(np_dt)
    mlin = np.tile(lin, (1, B, 1)).reshape(N_CORES, R, BLOCK)
    mbias = np.tile(np.concatenate([b3, b4 - b3], axis=2),
                    (1, B, 1)).reshape(N_CORES, R, 2 * BLOCK)
    idn = np.eye(R, dtype=np_dt)

    in_maps = [
        {"xp": np.ascontiguousarray(xp[:, :, i * S:(i + 1) * S + 1]),
         "mlin": mlin[i], "mbias": mbias[i], "idn": idn}
        for i in range(N_CORES)
    ]

    nc = _compiled(S, nb, _dt_name, _prog_opts)
    from concourse.bass_utils import run_bass_kernel_spmd

    res = run_bass_kernel_spmd(nc, in_maps, list(range(N_CORES)),
                               **(_run_opts or {}))
    partials = np.stack([r["out"][0] for r in res.results])  # [N_CORES, 3]
    s_lin, s3, s4 = partials.sum(axis=0, dtype=np.float64)

    n = float(B * M)
    loss = s_lin + math.log(s3) - math.log(n) + math.log(s4) - math.log(n)
    assert np.isfinite(loss)
    out = np.float32(loss)
    if _run_opts:
        return out, res
    return out


# revision 15
# speedup vs baseline: 1.2888x; 1.0113x over previous
"""Trainium2 Bass kernel for nn_ContrastiveLoss (B=4, C=256, H=W=256).

Strategy
--------
The reference computes four families of per-position channel dot products
over columns of x viewed as [B, C, N] (N = H*W), then scalar reductions:

  fam1 (pos_sim): dot(x[:,:,pos[t]],  x[:,:,pos[t+P]])   t in [0,P)
  fam2 (neg_sim): dot(x[:,:,neg[t]],  x[:,:,neg[t+Ng]])  t in [0,Ng)
  fam3 (pn1):     dot(x[:,:,pos[t]],  x[:,:,neg[t]])     t in [0,M)
  fam4 (pn2):     dot(x[:,:,pos[t]],  x[:,:,neg[t]])     t in [M,2M)

Each column of x participates in at most two dot products, so the union of
the four pairings is a degree-<=2 graph = disjoint paths and even cycles.
The host walks those paths/cycles and emits columns in walk order; in the
permuted tensor xp every dot product is between ADJACENT columns.

Device pipeline (per core, S = E/8 edges, nb = S/512 blocks):
  - xp [B, C, S+1] fp16 streams into 8 resident SBUF tiles [128, S+1]
    (one per batch x channel-chunk), two DMAs each (split at the product
    slice boundary for finer pipelining).  Whole 16KB rows per descriptor
    keep the 16 DMA engines at peak (~26 B/ns each).
  - DVE computes shifted products p[i] = t[i]*t[i+1] in [128, 4096] slices
    at the 2x_1p rate (~2.3us each); two mid-stream slices go to GpSimd.
  - The tensor engine reduces products over C via a ones-staircase matmul:
    lhsT one-hot-column [128, nb] routes each 512-block's column-sum into
    PSUM row jj.  PSUM tiles are PER BATCH [nb, 512] (chunk 0/1
    accumulate), so each batch's d tensor completes as soon as its two
    tiles are processed and the scalar tail pipelines per batch.
  - Tail per batch (emitted one tile later to avoid head-of-line stalls on
    the DVE queue): ACT exp, then three fused scalar_tensor_tensor
    mask-multiply+accumulate ops produce (sum lin*d, sum m3*exp(d),
    sum m4*exp(d)) per PSUM row into rcat columns.
  - Final: ones-vector matmul column-sums rcat [nb, 12] -> [1, 12], copied
    and DMA'd out.  Host combines 8 cores x 4 batch groups.

exp() needs no max-subtraction: |d| < ~0.5 for this data regime, so
sum(exp(d)) is stable in fp32 (guarded by a finiteness assert on the host).
"""

import math
import sys

import numpy as np

if "/opt/trn_rl_repo" not in sys.path:  # harness runs from a fresh dir
    sys.path.insert(0, "/opt/trn_rl_repo")

B, C, N = 4, 256, 65536
N_CORES = 8
BLOCK = 512          # edges per PSUM block (= max fp32 matmul free dim)
CHUNKS = C // 128    # channel chunks of 128 partitions
UNIT_BLOCKS = 8      # 512-blocks per DVE product slice (4096 edges)


# ---------------------------------------------------------------- host prep

def _build_walk(y):
    """Column permutation + per-edge family labels (0 = junk/padding)."""
    y = np.asarray(y).reshape(-1)
    pos_idx = np.nonzero(y == 1)[0]
    neg_idx = np.nonzero(y == 0)[0]
    P = pos_idx.shape[0] // 2
    Ng = neg_idx.shape[0] // 2
    M = min(P, Ng)

    nP, nN = 2 * P, 2 * Ng
    V = nP + nN
    t_pos = np.arange(nP)
    t_neg = np.arange(nN)
    nbrA = np.empty(V, dtype=np.int64)
    nbrA[:nP] = np.where(t_pos < P, t_pos + P, t_pos - P)
    nbrA[nP:] = nP + np.where(t_neg < Ng, t_neg + Ng, t_neg - Ng)
    famA = np.empty(V, dtype=np.int8)
    famA[:nP] = 1
    famA[nP:] = 2
    nbrB = np.full(V, -1, dtype=np.int64)
    nbrB[:2 * M] = nP + t_pos[:2 * M]
    nbrB[nP:nP + 2 * M] = t_neg[:2 * M]
    famB = np.zeros(V, dtype=np.int8)
    famB[:M] = 3
    famB[M:2 * M] = 4
    famB[nP:nP + M] = 3
    famB[nP + M:nP + 2 * M] = 4

    visited = np.zeros(V, dtype=bool)
    order = np.empty(V + V // 4 + 16, dtype=np.int64)
    fams_l = np.empty(order.shape[0], dtype=np.int8)
    no = 0
    ne = 0

    def walk_from(v0, is_cycle):
        nonlocal no, ne
        if no > 0:
            fams_l[ne] = 0  # junk edge joining the previous component
            ne += 1
        v = v0
        use_A = True  # endpoints/cycle starts leave via their A edge
        order[no] = v
        no += 1
        visited[v] = True
        while True:
            if use_A:
                nxt, fam = nbrA[v], famA[v]
            else:
                nxt = nbrB[v]
                if nxt < 0:
                    return
                fam = famB[v]
            if visited[nxt]:
                if is_cycle and nxt == v0 and not use_A:
                    fams_l[ne] = fam
                    ne += 1
                    order[no] = v0  # close the cycle
                    no += 1
                return
            fams_l[ne] = fam
            ne += 1
            order[no] = nxt
            no += 1
            visited[nxt] = True
            v = nxt
            use_A = not use_A

    for v0 in np.nonzero(nbrB < 0)[0]:
        if not visited[v0]:
            walk_from(int(v0), is_cycle=False)
    for v0 in range(V):
        if not visited[v0]:
            walk_from(int(v0), is_cycle=True)

    n_real = int((fams_l[:ne] > 0).sum())
    assert n_real == P + Ng + 2 * M, (n_real, P + Ng + 2 * M)

    per = N_CORES * BLOCK
    E_pad = ((ne + per - 1) // per) * per
    V_pad = E_pad + 1
    fams = np.zeros(E_pad, dtype=np.int8)
    fams[:ne] = fams_l[:ne]
    vert = np.zeros(V_pad, dtype=np.int64)
    vert[:no] = order[:no]
    colmap = np.where(vert < nP, pos_idx[np.minimum(vert, nP - 1)],
                      neg_idx[np.maximum(vert - nP, 0)])
    return colmap, fams, P, Ng, M


def _units(nb, ub=UNIT_BLOCKS):
    """Product slices: (col_start, n_cols) covering S in ub*512-col chunks."""
    out = []
    for s in range(0, nb, ub):
        w = min(ub, nb - s) * BLOCK
        out.append((s * BLOCK, w))
    return out


# ------------------------------------------------------------- device program

def trace_program(nc, tc, ctx, S, nb, dt_in,
                  gp_units=(), prod_bufs=4):
    """Emit the per-core program.

    DRAM (per core): xp [B, C, S+1] dt_in; mlin [R, BLOCK] f32 (linear
    weights); mbias [R, 2*BLOCK] dt_in (additive exp biases b3 | b4-b3);
    idn [R, R] dt_in (identity); out [1, 3] f32 = (sum lin*(d+b3),
    sum e^(d+b3), sum e^(d+b4)).
    """
    import concourse.mybir as mybir

    f32 = mybir.dt.float32
    Alu = mybir.AluOpType
    Act = mybir.ActivationFunctionType
    R = B * nb  # d row layout: r = nb*b + jj
    units = _units(nb)

    xp = nc.dram_tensor("xp", [B, C, S + 1], dt_in, kind="ExternalInput").ap()
    mlin = nc.dram_tensor("mlin", [R, BLOCK], f32, kind="ExternalInput").ap()
    mbias = nc.dram_tensor("mbias", [R, 2 * BLOCK], dt_in,
                           kind="ExternalInput").ap()
    idn = nc.dram_tensor("idn", [R, R], dt_in, kind="ExternalInput").ap()
    out = nc.dram_tensor("out", [R, 3], f32, kind="ExternalOutput").ap()

    const_pool = ctx.enter_context(tc.tile_pool(name="const", bufs=1))
    xp_pool = ctx.enter_context(tc.tile_pool(name="xp", bufs=1))
    prod_pool = ctx.enter_context(tc.tile_pool(name="prod", bufs=prod_bufs))
    stat_pool = ctx.enter_context(tc.tile_pool(name="stat", bufs=1))
    psum_pool = ctx.enter_context(tc.tile_pool(name="psum", bufs=1, space="PSUM"))

    # Staircase selector: zo[:, R-1] = 1 (all partitions), else 0.
    # lhsT = zo[:, R-1-r : 2R-1-r] puts the ones-column at local col r, so
    # the matmul lands the 128-partition column-sum in PSUM row r (other
    # rows accumulate exact zeros) -- matmul PSUM outputs must start at
    # partition 0, so rows can't be addressed via the output AP.
    zo = const_pool.tile([128, 2 * R - 1], dt_in)
    nc.vector.memset(zo[:], 0.0)
    nc.vector.memset(zo[:, R - 1:R], 1.0)

    mlin_t = const_pool.tile([R, BLOCK], f32)
    mbias_t = const_pool.tile([R, 2 * BLOCK], dt_in)
    idn_t = const_pool.tile([R, R], dt_in)

    d_psum = psum_pool.tile([R, BLOCK], f32, tag="d")
    rcat = stat_pool.tile([R, 3], f32)

    tiles = [(b, ch) for b in range(B) for ch in range(CHUNKS)]
    i_mm = 0
    for i, (b, ch) in enumerate(tiles):
        # the last tile gets finer slices so the post-DMA tail is short
        units_i = _units(nb, UNIT_BLOCKS // 2) if i == len(tiles) - 1 else units
        t = xp_pool.tile([128, S + 1], dt_in, tag=f"xp{i}")
        for k, (cs, w) in enumerate(units_i):
            lo = cs + (1 if k > 0 else 0)
            nc.sync.dma_start(t[:, lo:cs + w + 1],
                              xp[b, 128 * ch:128 * (ch + 1), lo:cs + w + 1])
        if i == 0:  # small aux loads after the first xp triggers
            nc.sync.dma_start(mlin_t[:], mlin)
            nc.sync.dma_start(mbias_t[:], mbias)
            nc.sync.dma_start(idn_t[:], idn)
        for k, (cs, w) in enumerate(units_i):
            eng = nc.gpsimd if (i, k) in gp_units else nc.vector
            p = prod_pool.tile([128, w], dt_in, tag=f"p{w}", name=f"p{w}")
            eng.tensor_mul(p[:], t[:, cs:cs + w], t[:, cs + 1:cs + w + 1])
            for jj in range(cs // BLOCK, (cs + w) // BLOCK):
                r = nb * b + jj
                nc.tensor.matmul(
                    d_psum[:, :], zo[:, R - 1 - r:2 * R - 1 - r],
                    p[:, BLOCK * jj - cs:BLOCK * (jj + 1) - cs],
                    start=(i_mm == 0), stop=False)
                i_mm += 1

    # tail: linear STT on the pure d first (the bias matmuls below mutate
    # d in PSUM), then d += b3 via identity-weighted matmul and
    # exp-accumulate on ACT; then d += (b4-b3) and exp-accumulate again.
    scr = stat_pool.tile([R, BLOCK], f32)
    junk = stat_pool.tile([R, BLOCK], f32)
    nc.vector.scalar_tensor_tensor(
        scr[:], d_psum[:], 1.0, mlin_t[:],
        Alu.mult, Alu.mult, accum_out=rcat[:, 0:1])
    nc.tensor.matmul(d_psum[:, :], idn_t[:], mbias_t[:, 0:BLOCK],
                     start=False, stop=False)
    nc.scalar.activation(junk[:], d_psum[:], Act.Exp,
                         accum_out=rcat[:, 1:2])
    nc.tensor.matmul(d_psum[:, :], idn_t[:], mbias_t[:, BLOCK:2 * BLOCK],
                     start=False, stop=True)
    nc.scalar.activation(junk[:], d_psum[:], Act.Exp,
                         accum_out=rcat[:, 2:3])

    nc.sync.dma_start(out, rcat[:])  # host sums the R rows


_CACHE = {}


def _compiled(S, nb, dt_name, prog_opts=None):
    key = (S, nb, dt_name, str(sorted((prog_opts or {}).items(),
                                      key=lambda kv: kv[0])))
    if key in _CACHE:
        return _CACHE[key]
    from contextlib import ExitStack

    import concourse.bacc as bacc
    import concourse.mybir as mybir
    import concourse.tile as tile

    dt_in = getattr(mybir.dt, dt_name)
    nc = bacc.Bacc("TRN2", target_bir_lowering=False, debug=False,
                   num_devices=N_CORES)
    with tile.TileContext(nc) as tc:
        with ExitStack() as ctx:
            trace_program(nc, tc, ctx, S, nb, dt_in, **(prog_opts or {}))
    nc.compile()
    _CACHE[key] = nc
    return nc


# -------------------------------------------------------------------- kernel

def kernel(x, y, _dt_name="float16", _run_opts=None, _prog_opts=None):
    x = np.asarray(x)
    y = np.asarray(y)
    assert x.shape == (B, C, 256, 256) and y.shape == (N,)

    colmap, fams, P, Ng, M = _build_walk(y)
    E = fams.shape[0]
    S = E // N_CORES
    nb = S // BLOCK
    assert nb * BLOCK * N_CORES == E and nb <= 128

    np_dt = {"float32": np.float32, "float16": np.float16,
             "bfloat16": np.float16}[_dt_name]
    x3 = x.reshape(B, C, N)
    xp = np.ascontiguousarray(x3[:, :, colmap], dtype=np_dt)  # [B, C, E+1]

    # per-core masks in d-row layout: row nb*b+jj covers edges
    # i*S+jj*512+[0,512), identical across the B batch rows
    R = B * nb
    fams_c = fams.reshape(N_CORES, nb, BLOCK)
    lin = np.where(fams_c == 1, -1.0 / (B * P),
                   np.where(fams_c == 2, -1.0 / (B * Ng), 0.0)).astype(np.float32)
    b3 = np.where(fams_c == 3, 0.0, -40.0).astype(np_dt)  # additive exp bias
    b4 = np.where(fams_c == 4, 0.0, -40.0).astype(np_dt)
    mlin = np.tile(lin, (1, B, 1)).reshape(N_CORES, R, BLOCK)
    mbias = np.tile(np.concatenate([b3, b4 - b3], axis=2),
                    (1, B, 1)).reshape(N_CORES, R, 2 * BLOCK)
    idn = np.eye(R, dtype=np_dt)

    in_maps = [
        {"xp": np.ascontiguousarray(xp[:, :, i * S:(i + 1) * S + 1]),
         "mlin": mlin[i], "mbias": mbias[i], "idn": idn}
        for i in range(N_CORES)
    ]

    nc = _compiled(S, nb, _dt_name, _prog_opts)
    from concourse.bass_utils import run_bass_kernel_spmd

    res = run_bass_kernel_spmd(nc, in_maps, list(range(N_CORES)),
                               **(_run_opts or {}))
    partials = np.stack([r["out"] for r in res.results])  # [N_CORES, R, 3]
    s_lin, s3, s4 = partials.sum(axis=(0, 1), dtype=np.float64)

    n = float(B * M)
    loss = s_lin + math.log(s3) - math.log(n) + math.log(s4) - math.log(n)
    assert np.isfinite(loss)
    out = np.float32(loss)
    if _run_opts:
        return out, res
    return out


# revision 17
# speedup vs baseline: 1.2983x; 1.0074x over previous
"""Trainium2 Bass kernel for nn_ContrastiveLoss (B=4, C=256, H=W=256).

Strategy
--------
The reference computes four families of per-position channel dot products
over columns of x viewed as [B, C, N] (N = H*W), then scalar reductions:

  fam1 (pos_sim): dot(x[:,:,pos[t]],  x[:,:,pos[t+P]])   t in [0,P)
  fam2 (neg_sim): dot(x[:,:,neg[t]],  x[:,:,neg[t+Ng]])  t in [0,Ng)
  fam3 (pn1):     dot(x[:,:,pos[t]],  x[:,:,neg[t]])     t in [0,M)
  fam4 (pn2):     dot(x[:,:,pos[t]],  x[:,:,neg[t]])     t in [M,2M)

Each column of x participates in at most two dot products, so the union of
the four pairings is a degree-<=2 graph = disjoint paths and even cycles.
The host walks those paths/cycles and emits columns in walk order; in the
permuted tensor xp every dot product is between ADJACENT columns.

Device pipeline (per core, S = E/8 edges, nb = S/512 blocks):
  - xp [B, C, S+1] fp16 streams into 8 resident SBUF tiles [128, S+1]
    (one per batch x channel-chunk), two DMAs each (split at the product
    slice boundary for finer pipelining).  Whole 16KB rows per descriptor
    keep the 16 DMA engines at peak (~26 B/ns each).
  - DVE computes shifted products p[i] = t[i]*t[i+1] in [128, 4096] slices
    at the 2x_1p rate (~2.3us each); two mid-stream slices go to GpSimd.
  - The tensor engine reduces products over C via a ones-staircase matmul:
    lhsT one-hot-column [128, nb] routes each 512-block's column-sum into
    PSUM row jj.  PSUM tiles are PER BATCH [nb, 512] (chunk 0/1
    accumulate), so each batch's d tensor completes as soon as its two
    tiles are processed and the scalar tail pipelines per batch.
  - Tail per batch (emitted one tile later to avoid head-of-line stalls on
    the DVE queue): ACT exp, then three fused scalar_tensor_tensor
    mask-multiply+accumulate ops produce (sum lin*d, sum m3*exp(d),
    sum m4*exp(d)) per PSUM row into rcat columns.
  - Final: ones-vector matmul column-sums rcat [nb, 12] -> [1, 12], copied
    and DMA'd out.  Host combines 8 cores x 4 batch groups.

exp() needs no max-subtraction: |d| < ~0.5 for this data regime, so
sum(exp(d)) is stable in fp32 (guarded by a finiteness assert on the host).
"""

import math
import sys

import numpy as np

if "/opt/trn_rl_repo" not in sys.path:  # harness runs from a fresh dir
    sys.path.insert(0, "/opt/trn_rl_repo")

B, C, N = 4, 256, 65536
N_CORES = 8
BLOCK = 512          # edges per PSUM block (= max fp32 matmul free dim)
CHUNKS = C // 128    # channel chunks of 128 partitions
UNIT_BLOCKS = 8      # 512-blocks per DVE product slice (4096 edges)


# ---------------------------------------------------------------- host prep

def _build_walk(y):
    """Column permutation + per-edge family labels (0 = junk/padding)."""
    y = np.asarray(y).reshape(-1)
    pos_idx = np.nonzero(y == 1)[0]
    neg_idx = np.nonzero(y == 0)[0]
    P = pos_idx.shape[0] // 2
    Ng = neg_idx.shape[0] // 2
    M = min(P, Ng)

    nP, nN = 2 * P, 2 * Ng
    V = nP + nN
    t_pos = np.arange(nP)
    t_neg = np.arange(nN)
    nbrA = np.empty(V, dtype=np.int64)
    nbrA[:nP] = np.where(t_pos < P, t_pos + P, t_pos - P)
    nbrA[nP:] = nP + np.where(t_neg < Ng, t_neg + Ng, t_neg - Ng)
    famA = np.empty(V, dtype=np.int8)
    famA[:nP] = 1
    famA[nP:] = 2
    nbrB = np.full(V, -1, dtype=np.int64)
    nbrB[:2 * M] = nP + t_pos[:2 * M]
    nbrB[nP:nP + 2 * M] = t_neg[:2 * M]
    famB = np.zeros(V, dtype=np.int8)
    famB[:M] = 3
    famB[M:2 * M] = 4
    famB[nP:nP + M] = 3
    famB[nP + M:nP + 2 * M] = 4

    visited = np.zeros(V, dtype=bool)
    order = np.empty(V + V // 4 + 16, dtype=np.int64)
    fams_l = np.empty(order.shape[0], dtype=np.int8)
    no = 0
    ne = 0

    def walk_from(v0, is_cycle):
        nonlocal no, ne
        if no > 0:
            fams_l[ne] = 0  # junk edge joining the previous component
            ne += 1
        v = v0
        use_A = True  # endpoints/cycle starts leave via their A edge
        order[no] = v
        no += 1
        visited[v] = True
        while True:
            if use_A:
                nxt, fam = nbrA[v], famA[v]
            else:
                nxt = nbrB[v]
                if nxt < 0:
                    return
                fam = famB[v]
            if visited[nxt]:
                if is_cycle and nxt == v0 and not use_A:
                    fams_l[ne] = fam
                    ne += 1
                    order[no] = v0  # close the cycle
                    no += 1
                return
            fams_l[ne] = fam
            ne += 1
            order[no] = nxt
            no += 1
            visited[nxt] = True
            v = nxt
            use_A = not use_A

    for v0 in np.nonzero(nbrB < 0)[0]:
        if not visited[v0]:
            walk_from(int(v0), is_cycle=False)
    for v0 in range(V):
        if not visited[v0]:
            walk_from(int(v0), is_cycle=True)

    n_real = int((fams_l[:ne] > 0).sum())
    assert n_real == P + Ng + 2 * M, (n_real, P + Ng + 2 * M)

    per = N_CORES * BLOCK
    E_pad = ((ne + per - 1) // per) * per
    V_pad = E_pad + 1
    fams = np.zeros(E_pad, dtype=np.int8)
    fams[:ne] = fams_l[:ne]
    vert = np.zeros(V_pad, dtype=np.int64)
    vert[:no] = order[:no]
    colmap = np.where(vert < nP, pos_idx[np.minimum(vert, nP - 1)],
                      neg_idx[np.maximum(vert - nP, 0)])
    return colmap, fams, P, Ng, M


def _units(nb, ub=UNIT_BLOCKS):
    """Product slices: (col_start, n_cols) covering S in ub*512-col chunks."""
    out = []
    for s in range(0, nb, ub):
        w = min(ub, nb - s) * BLOCK
        out.append((s * BLOCK, w))
    return out


# ------------------------------------------------------------- device program

def trace_program(nc, tc, ctx, S, nb, dt_in,
                  gp_units=(), prod_bufs=4):
    """Emit the per-core program.

    DRAM (per core): xp [B, C, S+1] dt_in; mlin [R, BLOCK] f32 (linear
    weights); mbias [R, 2*BLOCK] dt_in (additive exp biases b3 | b4-b3);
    idn [R, R] dt_in (identity); out [1, 3] f32 = (sum lin*(d+b3),
    sum e^(d+b3), sum e^(d+b4)).
    """
    import concourse.mybir as mybir

    f32 = mybir.dt.float32
    Alu = mybir.AluOpType
    Act = mybir.ActivationFunctionType
    R = B * nb  # d row layout: r = nb*b + jj
    units = _units(nb)

    xp = nc.dram_tensor("xp", [B, C, S + 1], dt_in, kind="ExternalInput").ap()
    mlin = nc.dram_tensor("mlin", [R, BLOCK], f32, kind="ExternalInput").ap()
    mbias = nc.dram_tensor("mbias", [R, 2 * BLOCK], dt_in,
                           kind="ExternalInput").ap()
    idn = nc.dram_tensor("idn", [R, R], dt_in, kind="ExternalInput").ap()
    out = nc.dram_tensor("out", [R, 3], f32, kind="ExternalOutput").ap()

    const_pool = ctx.enter_context(tc.tile_pool(name="const", bufs=1))
    xp_pool = ctx.enter_context(tc.tile_pool(name="xp", bufs=1))
    prod_pool = ctx.enter_context(tc.tile_pool(name="prod", bufs=prod_bufs))
    stat_pool = ctx.enter_context(tc.tile_pool(name="stat", bufs=1))
    psum_pool = ctx.enter_context(tc.tile_pool(name="psum", bufs=1, space="PSUM"))

    # Staircase selector: zo[:, R-1] = 1 (all partitions), else 0.
    # lhsT = zo[:, R-1-r : 2R-1-r] puts the ones-column at local col r, so
    # the matmul lands the 128-partition column-sum in PSUM row r (other
    # rows accumulate exact zeros) -- matmul PSUM outputs must start at
    # partition 0, so rows can't be addressed via the output AP.
    zo = const_pool.tile([128, 2 * R - 1], dt_in)
    nc.vector.memset(zo[:], 0.0)
    nc.vector.memset(zo[:, R - 1:R], 1.0)

    mlin_t = const_pool.tile([R, BLOCK], f32)
    mbias_t = const_pool.tile([R, 2 * BLOCK], dt_in)
    idn_t = const_pool.tile([R, R], dt_in)

    d_psum = psum_pool.tile([R, BLOCK], f32, tag="d")
    rcat = stat_pool.tile([R, 3], f32)

    tiles = [(b, ch) for b in range(B) for ch in range(CHUNKS)]
    i_mm = 0
    for i, (b, ch) in enumerate(tiles):
        # the last tile gets finer slices so the post-DMA tail is short;
        # earlier tiles load whole rows in one DMA (fewer queue bubbles).
        # The first tile goes through the GpSimd trigger path, whose
        # preamble finishes earlier than SP's, to start the stream sooner.
        last = i == len(tiles) - 1
        units_i = _units(nb, UNIT_BLOCKS // 2) if last else units
        t = xp_pool.tile([128, S + 1], dt_in, tag=f"xp{i}")
        trig = nc.sync
        if last:
            for k, (cs, w) in enumerate(units_i):
                lo = cs + (1 if k > 0 else 0)
                trig.dma_start(t[:, lo:cs + w + 1],
                               xp[b, 128 * ch:128 * (ch + 1), lo:cs + w + 1])
        else:
            trig.dma_start(t[:], xp[b, 128 * ch:128 * (ch + 1), :])
        if i == 0:  # small aux loads after the first xp triggers
            nc.sync.dma_start(mlin_t[:], mlin)
            nc.sync.dma_start(mbias_t[:], mbias)
            nc.sync.dma_start(idn_t[:], idn)
        for k, (cs, w) in enumerate(units_i):
            eng = nc.gpsimd if (i, k) in gp_units else nc.vector
            p = prod_pool.tile([128, w], dt_in, tag=f"p{w}", name=f"p{w}")
            eng.tensor_mul(p[:], t[:, cs:cs + w], t[:, cs + 1:cs + w + 1])
            for jj in range(cs // BLOCK, (cs + w) // BLOCK):
                r = nb * b + jj
                nc.tensor.matmul(
                    d_psum[:, :], zo[:, R - 1 - r:2 * R - 1 - r],
                    p[:, BLOCK * jj - cs:BLOCK * (jj + 1) - cs],
                    start=(i_mm == 0), stop=False)
                i_mm += 1

    # tail: linear STT on the pure d first (the bias matmuls below mutate
    # d in PSUM), then d += b3 via identity-weighted matmul and
    # exp-accumulate on ACT; then d += (b4-b3) and exp-accumulate again.
    scr = stat_pool.tile([R, BLOCK], f32)
    junk = stat_pool.tile([R, BLOCK], f32)
    nc.vector.scalar_tensor_tensor(
        scr[:], d_psum[:], 1.0, mlin_t[:],
        Alu.mult, Alu.mult, accum_out=rcat[:, 0:1])
    nc.tensor.matmul(d_psum[:, :], idn_t[:], mbias_t[:, 0:BLOCK],
                     start=False, stop=False)
    nc.scalar.activation(junk[:], d_psum[:], Act.Exp,
                         accum_out=rcat[:, 1:2])
    nc.tensor.matmul(d_psum[:, :], idn_t[:], mbias_t[:, BLOCK:2 * BLOCK],
                     start=False, stop=True)
    nc.scalar.activation(junk[:], d_psum[:], Act.Exp,
                         accum_out=rcat[:, 2:3])

    nc.sync.dma_start(out, rcat[:])  # host sums the R rows


_CACHE = {}


def _compiled(S, nb, dt_name, prog_opts=None):
    key = (S, nb, dt_name, str(sorted((prog_opts or {}).items(),
                                      key=lambda kv: kv[0])))
    if key in _CACHE:
        return _CACHE[key]
    from contextlib import ExitStack

    import concourse.bacc as bacc
    import concourse.mybir as mybir
    import concourse.tile as tile

    dt_in = getattr(mybir.dt, dt_name)
    nc = bacc.Bacc("TRN2", target_bir_lowering=False, debug=False,
                   num_devices=N_CORES)
    with tile.TileContext(nc) as tc:
        with ExitStack() as ctx:
            trace_program(nc, tc, ctx, S, nb, dt_in, **(prog_opts or {}))
    nc.compile()
    _CACHE[key] = nc
    return nc


# -------------------------------------------------------------------- kernel

def kernel(x, y, _dt_name="float16", _run_opts=None, _prog_opts=None):
    x = np.asarray(x)
    y = np.asarray(y)
    assert x.shape == (B, C, 256, 256) and y.shape == (N,)

    colmap, fams, P, Ng, M = _build_walk(y)
    E = fams.shape[0]
    S = E // N_CORES
    nb = S // BLOCK
    assert nb * BLOCK * N_CORES == E and nb <= 128

    np_dt = {"float32": np.float32, "float16": np.float16,
             "bfloat16": np.float16}[_dt_name]
    x3 = x.reshape(B, C, N)
    xp = np.ascontiguousarray(x3[:, :, colmap], dtype=np_dt)  # [B, C, E+1]

    # per-core masks in d-row layout: row nb*b+jj covers edges
    # i*S+jj*512+[0,512), identical across the B batch rows
    R = B * nb
    fams_c = fams.reshape(N_CORES, nb, BLOCK)
    lin = np.where(fams_c == 1, -1.0 / (B * P),
                   np.where(fams_c == 2, -1.0 / (B * Ng), 0.0)).astype(np.float32)
    b3 = np.where(fams_c == 3, 0.0, -40.0).astype(np_dt)  # additive exp bias
    b4 = np.where(fams_c == 4, 0.0, -40.0).astype(np_dt)
    mlin = np.tile(lin, (1, B, 1)).reshape(N_CORES, R, BLOCK)
    mbias = np.tile(np.concatenate([b3, b4 - b3], axis=2),
                    (1, B, 1)).reshape(N_CORES, R, 2 * BLOCK)
    idn = np.eye(R, dtype=np_dt)

    in_maps = [
        {"xp": np.ascontiguousarray(xp[:, :, i * S:(i + 1) * S + 1]),
         "mlin": mlin[i], "mbias": mbias[i], "idn": idn}
        for i in range(N_CORES)
    ]

    nc = _compiled(S, nb, _dt_name, _prog_opts)
    from concourse.bass_utils import run_bass_kernel_spmd

    res = run_bass_kernel_spmd(nc, in_maps, list(range(N_CORES)),
                               **(_run_opts or {}))
    partials = np.stack([r["out"] for r in res.results])  # [N_CORES, R, 3]
    s_lin, s3, s4 = partials.sum(axis=(0, 1), dtype=np.float64)

    n = float(B * M)
    loss = s_lin + math.log(s3) - math.log(n) + math.log(s4) - math.log(n)
    assert np.isfinite(loss)
    out = np.float32(loss)
    if _run_opts:
        return out, res
    return out
